# revision 49
# baseline (speedup 1.0000x reference)
"""Paged sparse-attention kernel for TRN2, head-sharded across 8 NeuronCores.

Key structural fact baked in from the reference computation: the mask
`triu(ones(q_len, kv_len), k=1)` with q_len=1024 masks every kv column
j > i for all queries i <= 1023, so only the first 1024 past-cache tokens
(= first 64 pages of the block table) ever contribute, and the fresh k/v
tensors are always fully masked.  The kernel therefore gathers 64 pages of
K/V per kv-head and runs causal attention of 1024 queries against those
1024 keys.  Softmax denominators come for free from a ones-column appended
to V inside the PV matmul.

Sharding: q is distributed per-core in [head*dim, tok] (d-major) layout --
the layout a fused QKV projection produces on device -- so no on-device
transpose of Q is needed.  K pages are gathered and transposed on device.
"""

import numpy as np

import concourse.bass as bass
import concourse.bacc as bacc
import concourse.mybir as mybir
from concourse.tile import TileContext
from concourse.bass_utils import run_bass_kernel_spmd

# problem constants (hardcoded per task instructions)
NCORES = 8
NUM_HEADS = 32
HPC = NUM_HEADS // NCORES          # 4 query heads per core
D = 128                            # head dim
PAGE = 16                          # page size
NPAGES = 512
TQ = 1024                          # query tokens
NKT = 8                            # k-token tiles of 128 that survive the mask
SCALE = 0.08838834764831845

f32 = mybir.dt.float32
bf16 = mybir.dt.bfloat16
EXP = mybir.ActivationFunctionType.Exp

_NC_CACHE: dict = {}


def build_bass(bt64: tuple) -> bass.Bass:
    """Build the SPMD per-core graph. bt64 = first 64 page ids (static)."""
    nc = bacc.Bacc(None, target_bir_lowering=False)
    q_ext = nc.declare_dram_parameter("q", [HPC * D, TQ], f32, isOutput=False)
    # per-core cache shard, token-row interleaved: [page, row, k/v, d]
    kv_ext = nc.declare_dram_parameter("kvc", [NPAGES, PAGE, 2, D], f32, isOutput=False)
    tri_ext = nc.declare_dram_parameter("tri", [128, 128], f32, isOutput=False)
    eye_ext = nc.declare_dram_parameter("eye", [128, 128], f32, isOutput=False)
    gidx_ext = nc.declare_dram_parameter("gidx", [128, NKT], mybir.dt.int32,
                                         isOutput=False)
    out_ext = nc.declare_dram_parameter("out", [TQ, HPC * D], f32, isOutput=True)

    # coalesce consecutive-page runs within each 8-page k-tile group
    tile_runs = []                      # t -> list of (row0, page0, L)
    for t in range(NKT):
        seg = bt64[8 * t:8 * t + 8]
        runs = []
        row0, pg0, L = 0, seg[0], 1
        for i in range(1, 8):
            if seg[i] == seg[i - 1] + 1:
                L += 1
            else:
                runs.append((row0, pg0, L))
                row0, pg0, L = i, seg[i], 1
        runs.append((row0, pg0, L))
        tile_runs.append(runs)

    with TileContext(nc) as tc:
        with tc.tile_pool(name="big", bufs=1) as big, \
             tc.tile_pool(name="stage", bufs=3) as stage, \
             tc.tile_pool(name="spsum", bufs=2, space="PSUM") as spsum, \
             tc.tile_pool(name="tpsum", bufs=1, space="PSUM") as tpsum, \
             tc.tile_pool(name="opsum", bufs=3, space="PSUM") as opsum:

            # ---- static SBUF tensors (all matmul operands in bf16) ----
            KT = big.tile([128, TQ], bf16, name="KT", tag="KT")          # [d, k-tok]
            QT = [big.tile([128, TQ], bf16, name=f"QT{h}", tag=f"QT{h}")
                  for h in range(HPC)]                                    # [d, q-tok]
            V16 = [big.tile([128, 130], bf16, name=f"V16_{t}", tag=f"V16_{t}")
                   for t in range(NKT)]                                   # [k-tok, d | 1]
            tri16 = big.tile([128, 128], bf16, name="tri16", tag="tri16")
            PT = {}                                                       # (h,c,t) -> [k,qchunk]

            dma_engines = [nc.sync]    # keep the Activation sequencer free for EXP
            eng_rr = [0]

            def next_eng():
                e = dma_engines[eng_rr[0] % len(dma_engines)]
                eng_rr[0] += 1
                return e

            # ---- constants (gidx early; eye/tri deferred into the ring) ----
            gidx = big.tile([128, NKT], mybir.dt.int32, name="gidx", tag="gidx")
            nc.sync.dma_start(out=gidx[:], in_=gidx_ext[:, :])
            eye = big.tile([128, 128], f32, name="eye", tag="eye")

            def load_eye():
                nc.sync.dma_start(out=eye[:], in_=eye_ext[:, :])

            trif = stage.tile([128, 128], f32, name="trif", tag="trif", bufs=1)

            def load_tri():
                nc.sync.dma_start(out=trif[:], in_=tri_ext[:, :])
                nc.vector.tensor_copy(tri16[:], trif[:])

            kv_view = kv_ext[:].rearrange("a b c d -> (a b) (c d)")

            # ---- load Q via SWDGE casting DMAs (f32 -> bf16 in flight) ----
            def load_q():
                for h in range(HPC):
                    nc.gpsimd.dma_start(out=QT[h][:],
                                        in_=q_ext[128 * h:128 * h + 128, :])

            kvf_tiles = {}

            def kv_load(t, direct):
                kvf = stage.tile([128, 2 * D], f32, name=f"kvf{t}", tag=f"kvf{t}",
                                 bufs=1)
                kvf_tiles[t] = kvf
                if direct:
                    # HWDGE coalesced page-runs: lowest latency to first tile
                    for row0, pg0, L in tile_runs[t]:
                        next_eng().dma_start(
                            out=kvf[16 * row0:16 * (row0 + L), :],
                            in_=kv_ext[pg0:pg0 + L])
                else:
                    # single SWDGE indirect gather: cheapest issue cost
                    nc.gpsimd.indirect_dma_start(
                        out=kvf[:],
                        out_offset=None,
                        in_=kv_view,
                        in_offset=bass.IndirectOffsetOnAxis(ap=gidx[:, t:t + 1],
                                                            axis=0))

            def kv_prep(t):
                kvf = kvf_tiles[t]
                # transpose K on the PE (f32), cast to bf16 on eviction
                ktp = tpsum.tile([128, 128], f32, name=f"ktp{t}", tag="ktp")
                nc.tensor.transpose(ktp[:], kvf[:, 0:128], eye[:])
                nc.vector.tensor_copy(KT[:, 128 * t:128 * t + 128], ktp[:])
                nc.vector.tensor_copy(V16[t][:, 0:128], kvf[:, 128:256])
                nc.vector.memset(V16[t][:, 128:129], 1.0)

            # ---- attention ----
            def phase1_merged(heads):
                # k-tiles 0..3: one [128,1024] S^T region per (h,t) covering
                # both q-chunks; valid columns are the contiguous [128t:1024].
                for t in range(4):
                    for h in heads:
                        st = spsum.tile([128, 1024], f32, name=f"st{h}_m{t}",
                                        tag="st")
                        nc.tensor.matmul(
                            st[:, 0:512],
                            lhsT=KT[:, 128 * t:128 * t + 128],
                            rhs=QT[h][:, 0:512], start=True, stop=True)
                        nc.tensor.matmul(
                            st[:, 512:1024],
                            lhsT=KT[:, 128 * t:128 * t + 128],
                            rhs=QT[h][:, 512:1024], start=True, stop=True)
                        pt = big.tile([128, 1024], bf16, name=f"pt{h}_m{t}",
                                      tag=f"pt{h}_m{t}")
                        PT[(h, 0, t)] = pt
                        PT[(h, 1, t)] = pt[:, 512:1024]
                        sl = slice(128 * t, 1024)
                        nc.scalar.activation(pt[:, sl], st[:, sl], EXP, scale=SCALE)
                        nc.gpsimd.tensor_mul(
                            pt[:, 128 * t:128 * t + 128],
                            pt[:, 128 * t:128 * t + 128], tri16[:])

            def phase1(c, heads):
                # S^T blocks [k-tile 128, q-chunk 512]; exp fused with scale;
                # diagonal blocks triangle-masked after exp (on gpsimd).
                # t-major: one KT stationary serves all heads' matmuls.
                for t in range(4 * c, 4 * c + 4):
                    for h in heads:
                        st = spsum.tile([128, 1024], f32, name=f"st{h}_{c}_{t}",
                                        tag="st")
                        nc.tensor.matmul(
                            st[:, 0:512],
                            lhsT=KT[:, 128 * t:128 * t + 128],
                            rhs=QT[h][:, 512 * c:512 * c + 512],
                            start=True, stop=True)
                        pt = big.tile([128, 512], bf16, name=f"pt{h}_{c}_{t}",
                                      tag=f"pt{h}_{c}_{t}")
                        PT[(h, c, t)] = pt
                        m = t - 4 * c
                        sl = slice(128 * m, 512)
                        nc.scalar.activation(pt[:, sl], st[:, sl], EXP, scale=SCALE)
                        nc.gpsimd.tensor_mul(
                            pt[:, 128 * m:128 * m + 128],
                            pt[:, 128 * m:128 * m + 128], tri16[:])

            def phase2(c):
                # O[q-sub, d|r] = sum_t PT[t]^T-slice @ [V|1]; normalize; store.
                for i in range(4 * c, 4 * c + 4):
                    m = i % 4
                    ostage = big.tile([128, HPC * D], f32, name=f"ost{i}", tag=f"ost{i}")
                    for h in range(HPC):
                        op = opsum.tile([128, 129], f32, name=f"op{i}_{h}", tag="op")
                        for t in range(i + 1):
                            nc.tensor.matmul(
                                op[:],
                                lhsT=PT[(h, c, t)][:, 128 * m:128 * m + 128],
                                rhs=V16[t][:, 0:129],
                                start=(t == 0), stop=(t == i))
                        rinv = stage.tile([128, 1], f32, name=f"rinv{i}_{h}", tag="rinv")
                        nc.vector.reciprocal(rinv[:], op[:, 128:129])
                        nc.vector.tensor_scalar_mul(
                            ostage[:, 128 * h:128 * h + 128], op[:, 0:128], rinv[:])
                    next_eng().dma_start(out=out_ext[128 * i:128 * i + 128, :],
                                         in_=ostage[:])

            # program order tuned so PE can start early and stays fed
            load_eye()
            load_q()                       # SWDGE ring: q first (needed early)
            for t in range(4, 8):
                kv_load(t, direct=False)   # then the indirect gathers
            for t in range(4):
                kv_load(t, direct=True)    # HWDGE rings carry only these
                kv_prep(t)
            load_tri()
            phase1_merged(range(HPC))
            for t in range(4, 8):
                kv_prep(t)
            phase2(0)
            phase1(1, range(HPC))
            phase2(1)

    return nc


def _get_nc(bt64: tuple) -> bass.Bass:
    if bt64 not in _NC_CACHE:
        nc = build_bass(bt64)
        nc.finalize()
        _NC_CACHE[bt64] = nc
    return _NC_CACHE[bt64]


def run(q, k, v, kv_cache, block_table, trace=False):
    q = np.asarray(q, dtype=np.float32)
    kv_cache = np.asarray(kv_cache, dtype=np.float32)
    bt = np.asarray(block_table).astype(np.int64)
    bt64 = tuple(int(x) for x in bt[:64])
    nc = _get_nc(bt64)
    tri = np.triu(np.ones((128, 128), np.float32))
    eye = np.eye(128, dtype=np.float32)
    p = np.arange(128)
    gidx = np.stack([np.asarray(bt64)[8 * t + p // 16] * PAGE + p % 16
                     for t in range(NKT)], axis=1).astype(np.int32)
    in_maps = []
    for i in range(NCORES):
        in_maps.append({
            "q": np.ascontiguousarray(q[:, HPC * D * i:HPC * D * (i + 1)].T),
            "kvc": np.ascontiguousarray(kv_cache[:, :, i].transpose(0, 2, 1, 3)),
            "tri": tri,
            "eye": eye,
            "gidx": gidx,
        })
    res = run_bass_kernel_spmd(nc, in_maps, list(range(NCORES)), trace=trace)
    out = np.concatenate([res.results[i]["out"] for i in range(NCORES)], axis=1)
    return out, res


def kernel(q, k, v, kv_cache, block_table):
    out, _ = run(q, k, v, kv_cache, block_table, trace=False)
    return out
